# revision 29
# baseline (speedup 1.0000x reference)
"""CAPE connectivity loss on 8 Trainium2 NeuronCores.

Math (reference): fg_prob = softmax(logits, ch_axis)[:, 1] = sigmoid(l1 - l0);
per batch, heat diffuses from 32 source voxels for 10 iterations of
  h = avg_pool3d_3x3x3(h) * prob;  h /= (max(h) + 1e-5)
then scores = h[endpoints_b], loss = mean over batches of (1 - mean(scores)).

Key structure exploited:
 1. The 3x3x3 box filter dilates support by exactly 1 voxel per iteration, so
    after 10 iterations heat is identically zero outside L-inf radius-10 balls
    around the 32 sources. Compute only on per-cluster regions (bbox+10),
    merged until the expanded regions are pairwise disjoint - then zero-BC
    diffusion per region is exact.
 2. The per-iteration max-normalization commutes with the linear
    pool*prob step: iterate u_{k+1} = pool(u_k)*prob unnormalized, record
    mu_k = max(u_k); then h_k = u_k / c_k with c_k = mu_k + SMOOTH*c_{k-1}.
    Zero-BC values outside a piece's owned claim only *underestimate* the true
    field, and the argmax voxel lies inside some owned claim, so the max over
    all region boxes equals the true global max.
 3. Oversized regions are split along an axis with a +10-voxel halo per
    internal cut; each piece's owned claim stays exact for all 10 iterations.

Device layout: pieces are rotated (largest axis -> partitions), their source
bboxes centered, and packed into columns of [128, ncols * boxH * boxW] bf16
SBUF tiles, in two width classes (21x21 "A" boxes and up to 34x34 "B" boxes)
with 1-voxel zero guard bands around each box. All per-iteration ops cover
only the dilated support band (sources +- k at iteration k). Per iteration:
the W 3-tap sum is two shifted tensor_tensor adds (bf16 2x mode); the
partition-axis (D) 3-tap sum is a PE matmul per column against a
block-tridiagonal 0/1 matrix (exact zero BC at block edges); for B the H
3-tap rides along on PE as 3 h-shifted matmuls accumulating into the same
PSUM bank, for A it runs as two more DVE adds; then one scalar_tensor_tensor
fuses u = (psum * 1/27) * prob. Since c_10 = mu_10 + eps*mu_9 + eps^2*mu_8 +
O(eps^3) with eps=1e-5, only u_8/u_9/u_10 maxima are reduced (ping-pong
buffers keep all three alive). Host combines per-core maxima, applies the
scale recurrence, and gathers the endpoint_b values.
"""

import ml_dtypes
import numpy as np

B, C, D, H, W = 2, 2, 128, 256, 256
N_PAIRS = 32
N_ITERS = 10
SMOOTH = 1e-5
R = N_ITERS  # diffusion reach in voxels
VOL = (D, H, W)

AXIS_CAP = 34   # max free-axis extent of a piece (split with halo beyond)
P_CAP = 128     # max partition-axis extent
A_FREE = 21     # class-A pieces have both free extents <= A_FREE
N_CORES = 8

# Results of the last device run (for test harness introspection).
LAST_RESULTS = None


# --------------------------------------------------------------------------
# planning: clusters -> regions -> pieces
# --------------------------------------------------------------------------

class Piece:
    __slots__ = ("lo", "hi", "clo", "chi", "srcs", "batch",
                 "perm", "cls", "core", "col", "p0", "offh", "offw")

    def __init__(self, lo, hi, clo, chi, srcs, batch):
        self.lo = np.asarray(lo); self.hi = np.asarray(hi)
        self.clo = np.asarray(clo); self.chi = np.asarray(chi)
        self.srcs = srcs
        self.batch = batch

    @property
    def ext(self):
        return self.hi - self.lo + 1


def _merge_clusters(pts):
    """Merge clusters until expanded regions (bbox+R) are pairwise disjoint."""
    clusters = [[i] for i in range(len(pts))]

    def bbox(c):
        p = pts[c]
        return p.min(0), p.max(0)

    changed = True
    while changed:
        changed = False
        out = []
        while clusters:
            c = clusters.pop()
            lo_c, hi_c = bbox(c)
            for k, o in enumerate(clusters):
                lo_o, hi_o = bbox(o)
                if np.all(lo_c - hi_o <= 2 * R) and np.all(lo_o - hi_c <= 2 * R):
                    clusters[k] = o + c
                    changed = True
                    break
            else:
                out.append(c)
        clusters = out
    return clusters


def _split(piece):
    """Split a piece until partition extent <= P_CAP and free extents <=
    AXIS_CAP. Claims are halved; each internal cut adds R halo."""
    out, done = [piece], []
    while out:
        p = out.pop()
        ext = p.ext
        order = np.argsort(-ext, kind="stable")
        ax = None
        if ext[order[0]] > P_CAP:
            ax = order[0]
        elif ext[order[1]] > AXIS_CAP:
            ax = order[1]
        elif ext[order[2]] > AXIS_CAP:
            ax = order[2]
        if ax is None:
            done.append(p)
            continue
        mid = (p.clo[ax] + p.chi[ax]) // 2
        for a, b in ((p.clo[ax], mid), (mid + 1, p.chi[ax])):
            nclo, nchi = p.clo.copy(), p.chi.copy()
            nclo[ax], nchi[ax] = a, b
            nlo, nhi = p.lo.copy(), p.hi.copy()
            nlo[ax] = max(a - R, p.lo[ax])
            nhi[ax] = min(b + R, p.hi[ax])
            srcs = [s for s in p.srcs if nlo[ax] <= s[ax] <= nhi[ax]]
            out.append(Piece(nlo, nhi, nclo, nchi, srcs, p.batch))
    return done


def make_pieces(ea):
    pieces = []
    for b in range(ea.shape[0]):
        pts = np.unique(ea[b], axis=0)
        for cl in _merge_clusters(pts):
            p = pts[cl]
            lo = np.maximum(p.min(0) - R, 0)
            hi = np.minimum(p.max(0) + R, np.asarray(VOL) - 1)
            base = Piece(lo, hi, lo, hi, [tuple(x) for x in p], b)
            pieces.extend(_split(base))
    return pieces


# --------------------------------------------------------------------------
# packing: pieces -> (class, core, col, partition offset)
# --------------------------------------------------------------------------

class Cfg:
    pass


def pack(pieces):
    """Assign each piece a rotation + (class, core, col, p0). Returns cfg."""
    for p in pieces:
        ext = p.ext
        p.perm = tuple(int(i) for i in np.argsort(-ext, kind="stable"))

    def free_ext(p):
        e = p.ext
        return e[p.perm[1]], e[p.perm[2]]

    cls_of = {}
    for p in pieces:
        fh, fw = free_ext(p)
        p.cls = 0 if (fh <= A_FREE and fw <= A_FREE) else 1
        cls_of.setdefault(p.cls, []).append(p)

    cfg = Cfg()
    cfg.classes = []
    col_list = []  # (cls, free_size, [pieces]) across all classes
    for cls in (0, 1):
        plist = cls_of.get(cls, [])
        if not plist:
            continue
        ih = max(int(free_ext(p)[0]) for p in plist)
        iw = max(int(free_ext(p)[1]) for p in plist)
        # first-fit decreasing bin pack by partition extent
        plist.sort(key=lambda p: -int(p.ext[p.perm[0]]))
        bins = []  # (used, [pieces])
        for p in plist:
            pe = int(p.ext[p.perm[0]])
            for b_ in bins:
                if b_[0] + pe <= 128:
                    p.p0 = b_[0]
                    b_[0] += pe
                    b_[1].append(p)
                    break
            else:
                p.p0 = 0
                bins.append([pe, [p]])
        # small-band classes do the H tap on DVE (1 matmul per chunk);
        # wide-band classes ride the H tap on PE as 3 shifted matmuls
        cfg.classes.append(dict(cls=cls, ih=ih, iw=iw, bh=ih + 2, bw=iw + 2,
                                cols=bins, dve_h=(cls == 0)))

    # Every core runs the same program over max-per-core column counts, so
    # the only thing that matters is minimizing cols-per-core: round-robin.
    for c in cfg.classes:
        for i, b_ in enumerate(c["cols"]):
            for p in b_[1]:
                p.core = i % N_CORES
                p.col = i // N_CORES
        c["j"] = (len(c["cols"]) + N_CORES - 1) // N_CORES

    # Center each piece's source bbox in its box (placement within the box is
    # free - zero BC surrounds the region either way), so that the dilated
    # support at iteration k is a tight band around the box center. Then
    # precompute per-class per-iteration op extents (box coords, h0:h1, w0:w1).
    for c in cfg.classes:
        ih, iw = c["ih"], c["iw"]
        ulo = [10**9, 10**9]
        uhi = [-1, -1]
        for b_ in c["cols"]:
            for p in b_[1]:
                offs = []
                for axi, ilen in ((1, ih), (2, iw)):
                    ax = p.perm[axi]
                    ext = int(p.ext[ax])
                    if p.srcs:
                        slo = min(s[ax] for s in p.srcs) - int(p.lo[ax])
                        shi = max(s[ax] for s in p.srcs) - int(p.lo[ax])
                    else:
                        slo = shi = ext // 2
                    # region start so src bbox center sits at interior center
                    start = 1 + (ilen - 1) // 2 - (slo + shi) // 2
                    start = min(max(start, 1), 1 + ilen - ext)
                    offs.append(start)
                    if p.srcs:
                        i01 = 0 if axi == 1 else 1
                        ulo[i01] = min(ulo[i01], start + slo)
                        uhi[i01] = max(uhi[i01], start + shi)
                p.offh, p.offw = offs
        if uhi[0] < 0:  # no sources in this class at all
            ulo, uhi = [1, 1], [ih, iw]
        c["ext_k"] = []
        for k in range(1, N_ITERS + 1):
            h0 = max(1, ulo[0] - k)
            h1 = min(1 + ih, uhi[0] + k + 1)
            w0 = max(1, ulo[1] - k)
            w1 = min(1 + iw, uhi[1] + k + 1)
            c["ext_k"].append((h0, h1, w0, w1))
    cfg.pieces = pieces
    return cfg


# --------------------------------------------------------------------------
# host-side data packing
# --------------------------------------------------------------------------

def build_inputs(cfg, logits, ea):
    """Build per-core input arrays. Returns in_maps (list of dicts)."""
    in_maps = [dict() for _ in range(N_CORES)]
    for ci, c in enumerate(cfg.classes):
        j, bh, bw = c["j"], c["bh"], c["bw"]
        for core in range(N_CORES):
            bf16 = ml_dtypes.bfloat16
            in_maps[core][f"l0_{ci}"] = np.zeros((128, j, bh, bw), bf16)
            # background prob must be sigmoid(l1-l0) ~= 0 so heat cannot leak
            # through box cells outside a (clipped) region slab - those cells
            # are beyond the volume edge, where the true BC is zero.
            in_maps[core][f"l1_{ci}"] = np.full((128, j, bh, bw), -100.0,
                                                bf16)
            in_maps[core][f"u0_{ci}"] = np.zeros((128, j, bh, bw), bf16)
            in_maps[core][f"tm_{ci}"] = np.zeros((128, j, 128), bf16)

    cls_idx = {c["cls"]: i for i, c in enumerate(cfg.classes)}
    for p in cfg.pieces:
        ci = cls_idx[p.cls]
        lo, hi, perm = p.lo, p.hi, p.perm
        pe = int(p.ext[perm[0]])
        eh = int(p.ext[perm[1]])
        ew = int(p.ext[perm[2]])
        sl = tuple(slice(int(lo[a]), int(hi[a]) + 1) for a in range(3))
        oh, ow = p.offh, p.offw
        for ch in (0, 1):
            blk = logits[p.batch, ch][sl].transpose(perm)
            arr = np.ascontiguousarray(blk).astype(ml_dtypes.bfloat16)
            in_maps[p.core][f"l{ch}_{ci}"][p.p0:p.p0 + pe, p.col,
                                           oh:oh + eh, ow:ow + ew] = arr
        u0 = in_maps[p.core][f"u0_{ci}"]
        for s in p.srcs:
            q = (s[perm[0]] - lo[perm[0]], s[perm[1]] - lo[perm[1]],
                 s[perm[2]] - lo[perm[2]])
            u0[p.p0 + q[0], p.col, oh + q[1], ow + q[2]] = 1.0
        tm = in_maps[p.core][f"tm_{ci}"]
        for i in range(pe):
            for d_ in (-1, 0, 1):
                if 0 <= i + d_ < pe:
                    tm[p.p0 + i, p.col, p.p0 + i + d_] = 1.0
    return in_maps


# --------------------------------------------------------------------------
# device kernel
# --------------------------------------------------------------------------

def build_nc(cfg):
    import concourse.bacc as bacc
    import concourse.tile as tile
    from concourse import mybir

    nc = bacc.Bacc("TRN2")
    dram = {}
    for ci, c in enumerate(cfg.classes):
        j, bh, bw = c["j"], c["bh"], c["bw"]
        for nm in ("l0", "l1", "u0"):
            dram[f"{nm}_{ci}"] = nc.dram_tensor(
                f"{nm}_{ci}", [128, j, bh, bw], mybir.dt.bfloat16,
                kind="ExternalInput")
        dram[f"tm_{ci}"] = nc.dram_tensor(
            f"tm_{ci}", [128, j, 128], mybir.dt.bfloat16, kind="ExternalInput")
        dram[f"out_{ci}"] = nc.dram_tensor(
            f"out_{ci}", [128, j, bh, bw], mybir.dt.bfloat16,
            kind="ExternalOutput")
    ncol_tot = sum(c["j"] for c in cfg.classes)
    dram["mx"] = nc.dram_tensor("mx", [128, 3, ncol_tot],
                                mybir.dt.float32, kind="ExternalOutput")

    with tile.TileContext(nc) as tc:
        with tc.tile_pool(name="sb", bufs=1) as sb, \
             tc.tile_pool(name="ps", bufs=6, space="PSUM") as pp:
            tiles = []
            for ci, c in enumerate(cfg.classes):
                j, bh, bw = c["j"], c["bh"], c["bw"]
                u = sb.tile([128, j, bh, bw], mybir.dt.bfloat16, tag=f"u{ci}")
                t1 = sb.tile([128, j, bh, bw], mybir.dt.bfloat16,
                             tag=f"t1{ci}")
                if c["dve_h"]:
                    t2 = sb.tile([128, j, bh, bw], mybir.dt.bfloat16,
                                 tag=f"t2{ci}", name=f"t2_{ci}")
                else:
                    t2 = None
                pr = sb.tile([128, j, bh, bw], mybir.dt.bfloat16,
                             tag=f"pr{ci}")
                ls = sb.tile([128, j, bh, bw], mybir.dt.bfloat16,
                             tag=f"ls{ci}")
                ub8 = sb.tile([128, j, bh, bw], mybir.dt.bfloat16,
                              tag=f"ub8{ci}")
                ub9 = sb.tile([128, j, bh, bw], mybir.dt.bfloat16,
                              tag=f"ub9{ci}")
                tm = sb.tile([128, j, 128], mybir.dt.bfloat16, tag=f"tm{ci}")
                tiles.append((u, t1, t2, pr, tm, ls, ub8, ub9))
            # u0/tm loads first: iteration 1's W pass only needs u and a
            # zeroed t1/t2, so compute starts while logits stream in.
            for ci, c in enumerate(cfg.classes):
                u = tiles[ci][0]
                tm = tiles[ci][4]
                nc.sync.dma_start(out=u[:], in_=dram[f"u0_{ci}"][:])
                nc.sync.dma_start(out=tm[:], in_=dram[f"tm_{ci}"][:])
            for ci, c in enumerate(cfg.classes):
                u, t1, t2, pr, tm, ls, ub8, ub9 = tiles[ci]
                # memsets on the otherwise-idle GpSimd engine, off DVE's path
                nc.gpsimd.memset(t1[:], 0.0)
                if t2 is not None:
                    nc.gpsimd.memset(t2[:], 0.0)
                nc.gpsimd.memset(ub8[:], 0.0)
                nc.gpsimd.memset(ub9[:], 0.0)
                nc.sync.dma_start(out=ls[:], in_=dram[f"l0_{ci}"][:])
                nc.sync.dma_start(out=pr[:], in_=dram[f"l1_{ci}"][:])

            mx = sb.tile([128, 3, ncol_tot], mybir.dt.float32, tag="mx")

            for it in range(N_ITERS):
                exts = [c["ext_k"][it] for c in cfg.classes]
                # u_k ping-pong: iters 8/9 write side buffers so that u_8,
                # u_9, u_10 all exist at the end; c_10 = mu_10 + eps*mu_9 +
                # eps^2*mu_8 + O(eps^3) needs only their maxima.
                def bufs(ci, it):
                    u, _, _, _, _, _, ub8, ub9 = tiles[ci]
                    rd = u if it <= 7 else (ub8 if it == 8 else ub9)
                    wr = u if it <= 6 else (ub8 if it == 7 else
                                            (ub9 if it == 8 else u))
                    return rd, wr
                # W pass for all classes first: t1 = rd(w-1) + rd(w+1) + rd
                for ci, c in enumerate(cfg.classes):
                    u, t1, t2, pr, tm, ls, ub8, ub9 = tiles[ci]
                    u, _ = bufs(ci, it)
                    h0, h1, w0, w1 = exts[ci]
                    nc.vector.tensor_add(t1[:, :, h0:h1, w0:w1],
                                         u[:, :, h0:h1, w0 - 1:w1 - 1],
                                         u[:, :, h0:h1, w0 + 1:w1 + 1])
                    nc.vector.tensor_add(t1[:, :, h0:h1, w0:w1],
                                         t1[:, :, h0:h1, w0:w1],
                                         u[:, :, h0:h1, w0:w1])
                    if c["dve_h"]:
                        # H pass on DVE: t2 = t1(h-1) + t1(h+1) + t1
                        nc.vector.tensor_add(t2[:, :, h0:h1, w0:w1],
                                             t1[:, :, h0 - 1:h1 - 1, w0:w1],
                                             t1[:, :, h0 + 1:h1 + 1, w0:w1])
                        nc.vector.tensor_add(t2[:, :, h0:h1, w0:w1],
                                             t2[:, :, h0:h1, w0:w1],
                                             t1[:, :, h0:h1, w0:w1])
                if it == 0:
                    # prob = sigmoid(l1 - l0), overlapped with iter-1 W pass
                    for ci, c in enumerate(cfg.classes):
                        pr, ls = tiles[ci][3], tiles[ci][5]
                        nc.vector.tensor_sub(pr[:], pr[:], ls[:])
                        nc.scalar.activation(
                            pr[:], pr[:], mybir.ActivationFunctionType.Sigmoid)
                # D pass on PE per column: psum accumulates the tridiagonal
                # T @ src (T also sums partition-axis neighbors); for classes
                # with dve_h=False the H tap rides along as 3 shifted
                # accumulating matmuls. Then u = (psum * 1/27) * prob.
                for ci, c in enumerate(cfg.classes):
                    u, t1, t2, pr, tm, ls, ub8, ub9 = tiles[ci]
                    _, u = bufs(ci, it)
                    h0, h1, w0, w1 = exts[ci]
                    wn = w1 - w0
                    rows = h1 - h0
                    nch = max(1, -(-(rows * wn) // 512))
                    rpc = -(-rows // nch)
                    while rpc * wn > 512:
                        nch += 1
                        rpc = -(-rows // nch)
                    src = t2 if c["dve_h"] else t1
                    shifts = (0,) if c["dve_h"] else (-1, 0, 1)
                    for jj in range(c["j"]):
                        for ch in range(nch):
                            r0 = h0 + ch * rpc
                            nr = min(rpc, h1 - r0)
                            ps = pp.tile([128, nr, wn], mybir.dt.float32,
                                         tag="ps")
                            for di, dh in enumerate(shifts):
                                nc.tensor.matmul(
                                    ps[:],
                                    tm[:, jj, :],
                                    src[:, jj, r0 + dh:r0 + dh + nr, w0:w1],
                                    start=(di == 0),
                                    stop=(di == len(shifts) - 1))
                            nc.vector.scalar_tensor_tensor(
                                out=u[:, jj, r0:r0 + nr, w0:w1],
                                in0=ps[:],
                                scalar=1.0 / 27.0,
                                in1=pr[:, jj, r0:r0 + nr, w0:w1],
                                op0=mybir.AluOpType.mult,
                                op1=mybir.AluOpType.mult)
                # per-column per-partition maxima of u_8/u_9/u_10 only:
                # c_10 = mu_10 + eps*mu_9 + eps^2*mu_8 + O(eps^3), eps=1e-5
                if it >= 7:
                    colbase = 0
                    for ci, c in enumerate(cfg.classes):
                        _, wr = bufs(ci, it)
                        h0, h1, w0, w1 = exts[ci]
                        nc.vector.tensor_reduce(
                            out=mx[:, it - 7, colbase:colbase + c["j"]],
                            in_=wr[:, :, h0:h1, w0:w1],
                            axis=mybir.AxisListType.XY,
                            op=mybir.AluOpType.max)
                        colbase += c["j"]

            for ci, c in enumerate(cfg.classes):
                nc.sync.dma_start(out=dram[f"out_{ci}"][:], in_=tiles[ci][0][:])
            nc.sync.dma_start(out=dram["mx"][:], in_=mx[:])
    nc.compile()
    return nc


# --------------------------------------------------------------------------
# host-side finalization
# --------------------------------------------------------------------------

def finalize(cfg, results, eb):
    """results: list of per-core dicts with out_{ci} and mx arrays."""
    cls_idx = {c["cls"]: i for i, c in enumerate(cfg.classes)}
    # mx column slot base per class
    col_base = {}
    s = 0
    for ci, c in enumerate(cfg.classes):
        col_base[ci] = s
        s += c["j"]

    # mx holds per-partition maxima of u_8, u_9, u_10; c_10 = mu_10 +
    # eps*mu_9 + eps^2*mu_8 + O(eps^3) with eps = SMOOTH = 1e-5, so the
    # truncation is ~1e-15 relative.
    mus = np.zeros((B, 3), dtype=np.float64)
    for p in cfg.pieces:
        ci = cls_idx[p.cls]
        pe = int(p.ext[p.perm[0]])
        m = results[p.core]["mx"][p.p0:p.p0 + pe, :, col_base[ci] + p.col]
        mus[p.batch] = np.maximum(mus[p.batch], m.max(axis=0))

    per_batch = []
    for b in range(B):
        cscale = 1.0
        for it in range(3):
            if mus[b, it] > 0:
                cscale = mus[b, it] + SMOOTH * cscale
        scores = []
        for e in eb[b]:
            val = 0.0
            for p in cfg.pieces:
                if p.batch != b:
                    continue
                if np.all(p.clo <= e) and np.all(e <= p.chi):
                    ci = cls_idx[p.cls]
                    q = (int(e[p.perm[0]] - p.lo[p.perm[0]]),
                         int(e[p.perm[1]] - p.lo[p.perm[1]]),
                         int(e[p.perm[2]] - p.lo[p.perm[2]]))
                    val = float(results[p.core][f"out_{ci}"]
                                [p.p0 + q[0], p.col,
                                 p.offh + q[1], p.offw + q[2]])
                    break
            scores.append(val / cscale)
        per_batch.append(1.0 - np.float32(np.mean(np.asarray(scores,
                                                             np.float32))))
    return np.array(np.mean(np.asarray(per_batch, np.float32)),
                    dtype=np.float32)


# --------------------------------------------------------------------------
# entry point
# --------------------------------------------------------------------------

def kernel(logits, labels, endpoints_a, endpoints_b):
    global LAST_RESULTS
    logits = np.asarray(logits)
    ea = np.asarray(endpoints_a).astype(np.int64)
    eb = np.asarray(endpoints_b).astype(np.int64)

    cfg = pack(make_pieces(ea))
    in_maps = build_inputs(cfg, logits, ea)
    nc = build_nc(cfg)

    from concourse.bass_utils import run_bass_kernel_spmd
    res = run_bass_kernel_spmd(nc, in_maps, core_ids=list(range(N_CORES)))
    LAST_RESULTS = res
    return finalize(cfg, res.results, eb)


if __name__ == "__main__":
    ins = {k: np.load(f"/tmp/in_{k}.npy")
           for k in ("logits", "labels", "endpoints_a", "endpoints_b")}
    out = kernel(**ins)
    print("kernel loss:", repr(out))
